# revision 11
# baseline (speedup 1.0000x reference)
"""Trainium2 Bass kernel for nn_FCNet_78898549227817.

Contract: kernel(**inputs) takes the FULL inputs from setup_inputs() and
returns the FULL output [2,12,12,12,47,1] float32.

Strategy:
  - 8 cores = (batch b in {0,1}) x (2x2 quadrant over X,Y). Each core gets a
    halo-padded [14,14,20] subvolume and computes its [6,6,12] output block
    independently (halo-redundant, no collectives).
  - On-device layout is [feature(partition), voxel(free)] throughout.
  - The per-voxel 47x47 regularized solves are replaced by a preconditioned
    Neumann series (H = (G + lam_n*0.5*P^T P)^{-1} precomputed on host),
    K=2 correction terms -> ~4e-6 relative error in exact arithmetic.
  - All matmuls run as float32r (full-rate fp32 on the PE at N>=256).
"""

import numpy as np

import concourse.bass as bass
import concourse.mybir as mybir
import concourse.tile as tile
from concourse import bacc
from concourse.bass_utils import run_bass_kernel_spmd

F32 = mybir.dt.float32
F32R = mybir.dt.float32r
AF = mybir.ActivationFunctionType

B = 2
NC = 47
D = 300
DPAD = 384  # 3 * 128
LAM_D = 1.0
LAM_N = 0.1
ALPHA = 50.0
SBAR = 0.5
KTERMS = 2
BN_EPS = 1e-5
IDX18 = list(range(16)) + [45, 46]

SUB = (14, 14, 20)  # per-core input subvolume
NVOX0 = SUB[0] * SUB[1] * SUB[2]  # 3920
VOLS = [(14, 14, 20), (12, 12, 18), (10, 10, 16), (8, 8, 14), (6, 6, 12)]
XGRP = {1: 2, 2: 2, 3: 4, 4: 6}  # x-planes per compute tile, per cascade
INIT_TILE = 490  # 3920 / 8

# 300 -> k-tile segments (partition count per k-tile)
KSEG = [(0, 128), (128, 128), (256, 44)]
# 300 -> m-tile segments for the P projection output
MSEG = [(0, 128), (128, 128), (256, 44)]

_CACHE = {}
LAST_RESULTS = None
TRACE = False


# --------------------------------------------------------------------------
# host-side data preparation
# --------------------------------------------------------------------------

def _kt_layout(mat, nkt):
    """[K, M] -> [128, nkt, M] with row r -> (p=r%128, kt=r//128), zero pad."""
    K, M = mat.shape
    out = np.zeros((128 * nkt, M), np.float64)
    out[:K] = mat
    return np.ascontiguousarray(
        out.reshape(nkt, 128, M).transpose(1, 0, 2)
    )


def _prep_shared(P, params):
    P = np.asarray(P, np.float64)
    data = {}
    for ci, layers in enumerate(params):
        fl = []
        for li, layer in enumerate(layers):
            if li < 3:
                W, bb, g, bt, m, v = [np.asarray(x, np.float64) for x in layer]
                s = g / np.sqrt(v + BN_EPS)
                Wf = W * s[:, None]
                bf = (bb - m) * s + bt
            else:
                W, bb = [np.asarray(x, np.float64) for x in layer]
                Wf, bf = W, bb
            fl.append((Wf, bf))
        (W1, b1), (W2, b2), (W3, b3), (W4, b4), (W5, b5) = fl
        data[f"w1_c{ci}"] = _kt_layout(W1.T, 10).astype(np.float32)  # [128,10,512]
        data[f"w2_c{ci}"] = _kt_layout(W2.T, 4).astype(np.float32)
        data[f"w3_c{ci}"] = _kt_layout(W3.T, 4).astype(np.float32)
        data[f"w4_c{ci}"] = _kt_layout(W4.T, 4).astype(np.float32)
        data[f"w5_c{ci}"] = _kt_layout(W5.T, 4).astype(np.float32)  # [128,4,47]
        for nm, bb_ in (("b1", b1), ("b2", b2), ("b3", b3), ("b4", b4)):
            data[f"{nm}_c{ci}"] = np.ascontiguousarray(
                bb_.reshape(4, 128).T
            ).astype(np.float32)  # [128,4]
        data[f"b5_c{ci}"] = b5.reshape(NC, 1).astype(np.float32)
    data["pt"] = P.T.astype(np.float32)  # [47, 300] lhsT for z = P c
    return data


def _prep_batch(AQ_b, P):
    A = np.asarray(AQ_b, np.float64)
    P = np.asarray(P, np.float64)
    G = A.T @ A + LAM_D * np.eye(NC)
    PtP = P.T @ P
    H = np.linalg.inv(G + LAM_N * SBAR * PtP)
    A18 = A[:, IDX18].T @ A[:, IDX18] + 0.01 * np.eye(18)
    Minit18 = A[:, IDX18] @ np.linalg.inv(A18)  # [300, 18] (A18 symmetric)
    Minit = np.zeros((D, NC), np.float64)
    Minit[:, :16] = Minit18[:, :16]
    Minit[:, 45:] = Minit18[:, 16:]
    return {
        "init_aq": _kt_layout(A, 3).astype(np.float32),        # [128,3,47]
        "init_c": _kt_layout(Minit, 3).astype(np.float32),     # [128,3,47]
        "nph": _kt_layout(-LAM_N * (P @ H), 3).astype(np.float32),
        "corr": (LAM_N * SBAR * (PtP @ H)).astype(np.float32),  # [47,47]
        "hmat": H.astype(np.float32),                           # [47,47]
    }


def _prep_bvox(b_block):
    """b_block [14,14,20,300] float -> [128, 3, 3920] k-tiled transpose."""
    bt = np.asarray(b_block, np.float32).reshape(NVOX0, D).T  # [300, 3920]
    out = np.zeros((DPAD, NVOX0), np.float32)
    out[:D] = bt
    return np.ascontiguousarray(out.reshape(3, 128, NVOX0).transpose(1, 0, 2))


# --------------------------------------------------------------------------
# device program
# --------------------------------------------------------------------------

def _straddle_segments(o):
    """feature rows [47*o, 47*o+47) -> list of (kt, p0, q0, length)."""
    f0 = 47 * o
    segs = []
    q = 0
    while q < 47:
        f = f0 + q
        kt, p = f // 128, f % 128
        length = min(47 - q, 128 - p)
        segs.append((kt, p, q, length))
        q += length
    return segs


def build_program():
    nc = bacc.Bacc("TRN2", target_bir_lowering=False, debug=False, num_devices=8)

    # ---- DRAM I/O ----
    bvt = nc.dram_tensor("bvt", [128, 3, NVOX0], F32R, kind="ExternalInput")
    init_aq = nc.dram_tensor("init_aq", [128, 3, NC], F32R, kind="ExternalInput")
    init_c = nc.dram_tensor("init_c", [128, 3, NC], F32R, kind="ExternalInput")
    pt_d = nc.dram_tensor("pt", [NC, D], F32R, kind="ExternalInput")
    nph_d = nc.dram_tensor("nph", [128, 3, NC], F32R, kind="ExternalInput")
    corr_d = nc.dram_tensor("corr", [NC, NC], F32R, kind="ExternalInput")
    hmat_d = nc.dram_tensor("hmat", [NC, NC], F32R, kind="ExternalInput")
    wdram = {}
    for ci in range(4):
        wdram[f"w1_c{ci}"] = nc.dram_tensor(f"w1_c{ci}", [128, 10, 512], F32R, kind="ExternalInput")
        for nm in ("w2", "w3", "w4"):
            wdram[f"{nm}_c{ci}"] = nc.dram_tensor(f"{nm}_c{ci}", [128, 4, 512], F32R, kind="ExternalInput")
        wdram[f"w5_c{ci}"] = nc.dram_tensor(f"w5_c{ci}", [128, 4, NC], F32R, kind="ExternalInput")
        for nm in ("b1", "b2", "b3", "b4"):
            wdram[f"{nm}_c{ci}"] = nc.dram_tensor(f"{nm}_c{ci}", [128, 4], F32, kind="ExternalInput")
        wdram[f"b5_c{ci}"] = nc.dram_tensor(f"b5_c{ci}", [NC, 1], F32, kind="ExternalInput")
    out_d = nc.dram_tensor("out", [NC, 432], F32, kind="ExternalOutput")

    from contextlib import ExitStack
    with tile.TileContext(nc) as tc, ExitStack() as ctx:
        consts = ctx.enter_context(tc.tile_pool(name="consts", bufs=1))
        wpool1 = ctx.enter_context(tc.tile_pool(name="wpool1", bufs=2))
        wpool = ctx.enter_context(tc.tile_pool(name="wpool", bufs=1))
        bpool = ctx.enter_context(tc.tile_pool(name="bpool", bufs=2))
        fpool = ctx.enter_context(tc.tile_pool(name="fpool", bufs=2))
        hpool = ctx.enter_context(tc.tile_pool(name="hpool", bufs=2))
        upool = ctx.enter_context(tc.tile_pool(name="upool", bufs=1))
        s2pool = ctx.enter_context(tc.tile_pool(name="s2pool", bufs=1))
        smalls = ctx.enter_context(tc.tile_pool(name="smalls", bufs=2))
        cvols = ctx.enter_context(tc.tile_pool(name="cvols", bufs=1))
        pspool = ctx.enter_context(tc.tile_pool(name="pspool", bufs=6, space="PSUM"))
        ps47 = ctx.enter_context(tc.tile_pool(name="ps47", bufs=2, space="PSUM"))

        # ---- persistent constants ----
        pt_sb = consts.tile([NC, D], F32R)
        nc.sync.dma_start(pt_sb[:], pt_d[:])
        nph_sb = consts.tile([128, 3, NC], F32R)
        nc.sync.dma_start(nph_sb[:], nph_d[:])
        corr_sb = consts.tile([NC, NC], F32R)
        nc.sync.dma_start(corr_sb[:], corr_d[:])
        hmat_sb = consts.tile([NC, NC], F32R)
        nc.sync.dma_start(hmat_sb[:], hmat_d[:])
        iaq_sb = consts.tile([128, 3, NC], F32R)
        nc.sync.dma_start(iaq_sb[:], init_aq[:])
        ic_sb = consts.tile([128, 3, NC], F32R)
        nc.sync.dma_start(ic_sb[:], init_c[:])

        # ---- volumes ----
        aqtb = cvols.tile([NC] + list(SUB), F32, tag="aqtb")
        aqtb_flat = aqtb[:].rearrange("p x y z -> p (x y z)")
        c0_t = cvols.tile([NC] + list(VOLS[0]), F32R, tag="c0", name="c0_t")
        cvol = {0: c0_t}
        c_flat0 = cvol[0][:].rearrange("p x y z -> p (x y z)")

        # ---- init: AQ_Tb and c_init ----
        for t in range(NVOX0 // INIT_TILE):
            v0 = t * INIT_TILE
            bt_sb = fpool.tile([128, 3, INIT_TILE], F32R, tag="feats")
            nc.sync.dma_start(bt_sb[:], bvt[:, :, v0:v0 + INIT_TILE])
            pa = pspool.tile([NC, INIT_TILE], F32, tag="ps")
            pc = pspool.tile([NC, INIT_TILE], F32, tag="ps")
            for kt in range(3):
                nc.tensor.matmul(
                    pa[:], iaq_sb[:, kt, :],
                    bt_sb[:, kt, :],
                    start=(kt == 0), stop=(kt == 2))
            for kt in range(3):
                nc.tensor.matmul(
                    pc[:], ic_sb[:, kt, :],
                    bt_sb[:, kt, :],
                    start=(kt == 0), stop=(kt == 2))
            nc.scalar.activation(aqtb_flat[:, v0:v0 + INIT_TILE], pa[:], AF.Copy)
            nc.scalar.activation(c_flat0[:, v0:v0 + INIT_TILE], pc[:], AF.Copy)

        # ---- cascades ----
        for n in range(1, 5):
            ci = n - 1
            Xi, Yi, Zi = VOLS[n - 1]
            Xo, Yo, Zo = VOLS[n]
            XG = XGRP[n]
            NT = XG * Yo * Zo
            c_in = cvol[n - 1]
            if n < 4:
                cvol[n] = cvols.tile([NC] + list(VOLS[n]), F32R, tag=f"c{n}",
                                     name=f"cvol{n}")

            # weights for this cascade
            w1 = wpool1.tile([128, 10, 512], F32R, tag="w1")
            nc.sync.dma_start(w1[:], wdram[f"w1_c{ci}"][:])
            w2 = wpool.tile([128, 4, 512], F32R, tag="w2")
            nc.sync.dma_start(w2[:], wdram[f"w2_c{ci}"][:])
            w3 = wpool.tile([128, 4, 512], F32R, tag="w3")
            nc.sync.dma_start(w3[:], wdram[f"w3_c{ci}"][:])
            w4 = wpool.tile([128, 4, 512], F32R, tag="w4")
            nc.sync.dma_start(w4[:], wdram[f"w4_c{ci}"][:])
            w5 = wpool.tile([128, 4, NC], F32R, tag="w5")
            nc.sync.dma_start(w5[:], wdram[f"w5_c{ci}"][:])
            bs = {}
            for nm in ("b1", "b2", "b3", "b4"):
                bs[nm] = bpool.tile([128, 4], F32, tag=nm, name=f"{nm}_{ci}")
                nc.sync.dma_start(bs[nm][:], wdram[f"{nm}_c{ci}"][:])
            b5 = bpool.tile([NC, 1], F32, tag="b5")
            nc.sync.dma_start(b5[:], wdram[f"b5_c{ci}"][:])

            for ti in range(Xo // XG):
                ox = ti * XG
                # ---------- gather neighbor features ----------
                ft = fpool.tile([128, 10, NT], F32R, tag="feats")
                YZ = Yo * Zo
                for o in range(27):
                    dx, dy, dz = o // 9, (o // 3) % 3, o % 3
                    for (kt, p0, q0, ln) in _straddle_segments(o):
                        for xp in range(XG):
                            nc.sync.dma_start(
                                ft[p0:p0 + ln, kt, xp * YZ:(xp + 1) * YZ],
                                c_in[q0:q0 + ln, ox + dx + xp,
                                     dy:dy + Yo, dz:dz + Zo])
                # ---------- MLP ----------
                h1 = hpool.tile([128, 4, NT], F32R, tag="h1")
                for mt in range(4):
                    pm = pspool.tile([128, NT], F32, tag="ps")
                    for kt in range(10):
                        if kt < 9:
                            lhs = w1[:, kt, mt * 128:(mt + 1) * 128]
                            rhs = ft[:, kt, :]
                        else:
                            lhs = w1[0:117, 9, mt * 128:(mt + 1) * 128]
                            rhs = ft[0:117, 9, :]
                        nc.tensor.matmul(pm[:], lhs, rhs,
                                         start=(kt == 0), stop=(kt == 9))
                    nc.scalar.activation(h1[:, mt, :], pm[:], AF.Relu,
                                         bias=bs["b1"][:, mt:mt + 1])
                hprev = h1
                for li, (wt, bname) in enumerate(
                        ((w2, "b2"), (w3, "b3"), (w4, "b4"))):
                    hnext = hpool.tile([128, 4, NT], F32R,
                                       tag="h2" if li % 2 == 0 else "h1")
                    for mt in range(4):
                        pm = pspool.tile([128, NT], F32, tag="ps")
                        for kt in range(4):
                            nc.tensor.matmul(
                                pm[:], wt[:, kt, mt * 128:(mt + 1) * 128],
                                hprev[:, kt, :],
                                start=(kt == 0), stop=(kt == 3))
                        nc.scalar.activation(hnext[:, mt, :], pm[:], AF.Relu,
                                             bias=bs[bname][:, mt:mt + 1])
                    hprev = hnext
                pw = ps47.tile([NC, NT], F32, tag="ps47")
                for kt in range(4):
                    nc.tensor.matmul(pw[:], w5[:, kt, :],
                                     hprev[:, kt, :],
                                     start=(kt == 0), stop=(kt == 3))
                # ---------- build rhs and current c ----------
                rhs_sb = smalls.tile([NC, NT], F32R, tag="rhs")
                nc.scalar.activation(rhs_sb[:], pw[:], AF.Identity, bias=b5[:, 0:1])
                c_view = c_in[:, ox + 1:ox + 1 + XG, 1:1 + Yo, 1:1 + Zo]
                if n > 1:
                    nc.vector.tensor_add(rhs_sb[:], rhs_sb[:], c_view)
                aq_view = aqtb[:, n + ox:n + ox + XG, n:n + Yo, n:n + Zo]
                nc.vector.tensor_add(rhs_sb[:], rhs_sb[:], aq_view)
                c_sb = smalls.tile([NC, NT], F32R, tag="cnew")
                nc.vector.tensor_copy(c_sb[:], c_view)
                # ---------- 3 data-consistency iterations ----------
                for it in range(3):
                    # s2 = Sigmoid(-alpha * P c)^2
                    s2t = s2pool.tile([128, 3, NT], F32, tag="s2")
                    for mi, (m0, ml) in enumerate(MSEG):
                        zp = pspool.tile([128, NT], F32, tag="ps")
                        nc.tensor.matmul(zp[0:ml, :],
                                         pt_sb[:, m0:m0 + ml],
                                         c_sb[:],
                                         start=True, stop=True)
                        nc.scalar.activation(s2t[0:ml, mi, :], zp[0:ml, :],
                                             AF.Sigmoid, scale=-ALPHA)
                        nc.scalar.activation(s2t[0:ml, mi, :], s2t[0:ml, mi, :],
                                             AF.Square)
                    # t0 = H rhs
                    tp = ps47.tile([NC, NT], F32, tag="ps47")
                    nc.tensor.matmul(tp[:], hmat_sb[:],
                                     rhs_sb[:], start=True, stop=True)
                    t_sb = smalls.tile([NC, NT], F32R, tag="t")
                    nc.scalar.activation(t_sb[:], tp[:], AF.Copy)
                    cn_sb = smalls.tile([NC, NT], F32R, tag="cnew")
                    nc.vector.tensor_copy(cn_sb[:], t_sb[:])
                    for k in range(KTERMS):
                        u_sb = upool.tile([128, 3, NT], F32R, tag="u")
                        for mi, (m0, ml) in enumerate(MSEG):
                            zp = pspool.tile([128, NT], F32, tag="ps")
                            nc.tensor.matmul(zp[0:ml, :],
                                             pt_sb[:, m0:m0 + ml],
                                             t_sb[:],
                                             start=True, stop=True)
                            nc.vector.tensor_mul(u_sb[0:ml, mi, :], zp[0:ml, :],
                                                 s2t[0:ml, mi, :])
                        tp2 = ps47.tile([NC, NT], F32, tag="ps47")
                        for ki, (k0, kl) in enumerate(KSEG):
                            nc.tensor.matmul(tp2[:],
                                             nph_sb[0:kl, ki, :],
                                             u_sb[0:kl, ki, :],
                                             start=(ki == 0), stop=False)
                        nc.tensor.matmul(tp2[:], corr_sb[:],
                                         t_sb[:],
                                         start=False, stop=True)
                        t_sb = smalls.tile([NC, NT], F32R, tag="t")
                        nc.scalar.activation(t_sb[:], tp2[:], AF.Copy)
                        nc.vector.tensor_add(cn_sb[:], cn_sb[:], t_sb[:])
                    c_sb = cn_sb
                # ---------- write result ----------
                if n < 4:
                    nc.vector.tensor_copy(cvol[n][:, ox:ox + XG, :, :], c_sb[:])
                else:
                    nc.sync.dma_start(out_d[:], c_sb[:].bitcast(F32))

    nc.compile()
    return nc


# --------------------------------------------------------------------------
# entry point
# --------------------------------------------------------------------------

def kernel(b, AQ, P, params, lambda_deep=1.0, lambda_neg=0.1, alpha=50.0):
    global LAST_RESULTS
    b = np.asarray(b, np.float32)
    AQ = np.asarray(AQ, np.float32)
    P_np = np.asarray(P, np.float32)

    shared = _prep_shared(P_np, params)
    batch = [_prep_batch(AQ[bi], P_np) for bi in range(B)]

    if "nc" not in _CACHE:
        _CACHE["nc"] = build_program()
    nc = _CACHE["nc"]

    in_maps = []
    coords = []
    for bi in range(B):
        for qx in range(2):
            for qy in range(2):
                x0, y0 = 6 * qx, 6 * qy
                blk = b[bi, x0:x0 + 14, y0:y0 + 14, :, :, 0]
                m = dict(shared)
                m.update(batch[bi])
                m["bvt"] = _prep_bvox(blk)
                in_maps.append(m)
                coords.append((bi, qx, qy))

    res = run_bass_kernel_spmd(nc, in_maps, core_ids=list(range(8)), trace=TRACE)
    LAST_RESULTS = res

    out = np.zeros((B, 12, 12, 12, NC, 1), np.float32)
    for i, (bi, qx, qy) in enumerate(coords):
        blk = res.results[i]["out"].reshape(NC, 6, 6, 12).transpose(1, 2, 3, 0)
        out[bi, 6 * qx:6 * qx + 6, 6 * qy:6 * qy + 6, :, :, 0] = blk
    return out


# revision 12
# speedup vs baseline: 2.2581x; 2.2581x over previous
"""Trainium2 Bass kernel for nn_FCNet_78898549227817.

Contract: kernel(**inputs) takes the FULL inputs from setup_inputs() and
returns the FULL output [2,12,12,12,47,1] float32.

Strategy:
  - 8 cores = (batch b in {0,1}) x (2x2 quadrant over X,Y). Each core gets a
    halo-padded [14,14,20] subvolume and computes its [6,6,12] output block
    independently (halo-redundant, no collectives).
  - Volumes live in SBUF as [coef(partition), Z, X, Y] (z-major). Each c
    volume is stored doubled on partitions: rows 47..93 hold the z+1-shifted
    copy (built by one cheap SBUF DMA per cascade, contiguous X*Y runs).
  - The 27-neighbor MLP input is never materialized: layer 1 runs as 18
    accumulating matmuls per tile (9 with K=94 covering dz in {0,1}, 9 with
    K=47 for dz=2) whose moving operands are strided views of the c volume.
  - The per-voxel 47x47 regularized solves are replaced by a preconditioned
    Neumann series (H = (G + lam_n*0.5*P^T P)^{-1} precomputed on host),
    K=2 correction terms -> ~4e-6 relative error in exact arithmetic.
  - All matmuls run as float32r (full-rate fp32 on the PE at N>=256).
"""

import numpy as np

import concourse.bass as bass
import concourse.mybir as mybir
import concourse.tile as tile
from concourse import bacc
from concourse.bass_utils import run_bass_kernel_spmd

F32 = mybir.dt.float32
F32R = mybir.dt.float32r
AF = mybir.ActivationFunctionType

B = 2
NC = 47
D = 300
DPAD = 384  # 3 * 128
LAM_D = 1.0
LAM_N = 0.1
ALPHA = 50.0
SBAR = 0.5
KTERMS = 2
BN_EPS = 1e-5
IDX18 = list(range(16)) + [45, 46]

SUB = (14, 14, 20)  # per-core input subvolume (X, Y, Z)
NVOX0 = SUB[0] * SUB[1] * SUB[2]  # 3920
# (X, Y, Z) per cascade stage; volume n is the input to cascade n+1
VOLS = [(14, 14, 20), (12, 12, 18), (10, 10, 16), (8, 8, 14), (6, 6, 12)]
ZGRP = {1: 3, 2: 4, 3: 7, 4: 12}  # output z-planes per compute tile
INIT_ZG = 2  # init tile = 2 z-planes of the full subvolume -> N=392

# 300 -> k-tile segments (partition count per k-tile)
KSEG = [(0, 128), (128, 128), (256, 44)]
MSEG = [(0, 128), (128, 128), (256, 44)]

_CACHE = {}
LAST_RESULTS = None
TRACE = False


# --------------------------------------------------------------------------
# host-side data preparation
# --------------------------------------------------------------------------

def _kt_layout(mat, nkt):
    """[K, M] -> [128, nkt, M] with row r -> (p=r%128, kt=r//128), zero pad."""
    K, M = mat.shape
    out = np.zeros((128 * nkt, M), np.float64)
    out[:K] = mat
    return np.ascontiguousarray(out.reshape(nkt, 128, M).transpose(1, 0, 2))


def _prep_shared(P, params):
    P = np.asarray(P, np.float64)
    data = {}
    for ci, layers in enumerate(params):
        fl = []
        for li, layer in enumerate(layers):
            if li < 3:
                W, bb, g, bt, m, v = [np.asarray(x, np.float64) for x in layer]
                s = g / np.sqrt(v + BN_EPS)
                Wf = W * s[:, None]
                bf = (bb - m) * s + bt
            else:
                W, bb = [np.asarray(x, np.float64) for x in layer]
                Wf, bf = W, bb
            fl.append((Wf, bf))
        (W1, b1), (W2, b2), (W3, b3), (W4, b4), (W5, b5) = fl
        W1T = W1.T  # [1269, 512]; feature f = 47*(9dx+3dy+dz)+q
        w94 = np.zeros((94, 9, 512), np.float64)
        w47 = np.zeros((NC, 9, 512), np.float64)
        for g in range(9):
            f0 = 47 * 3 * g  # rows for offsets (dx,dy,0..2)
            w94[:, g, :] = W1T[f0:f0 + 94]
            w47[:, g, :] = W1T[f0 + 94:f0 + 141]
        data[f"w94_c{ci}"] = np.ascontiguousarray(w94).astype(np.float32)
        data[f"w47_c{ci}"] = np.ascontiguousarray(w47).astype(np.float32)
        data[f"w2_c{ci}"] = _kt_layout(W2.T, 4).astype(np.float32)
        data[f"w3_c{ci}"] = _kt_layout(W3.T, 4).astype(np.float32)
        data[f"w4_c{ci}"] = _kt_layout(W4.T, 4).astype(np.float32)
        data[f"w5_c{ci}"] = _kt_layout(W5.T, 4).astype(np.float32)  # [128,4,47]
        for nm, bb_ in (("b1", b1), ("b2", b2), ("b3", b3), ("b4", b4)):
            data[f"{nm}_c{ci}"] = np.ascontiguousarray(
                bb_.reshape(4, 128).T
            ).astype(np.float32)  # [128,4]
        data[f"b5_c{ci}"] = b5.reshape(NC, 1).astype(np.float32)
    data["pt"] = P.T.astype(np.float32)  # [47, 300] lhsT for z = P c
    return data


def _prep_batch(AQ_b, P):
    A = np.asarray(AQ_b, np.float64)
    P = np.asarray(P, np.float64)
    G = A.T @ A + LAM_D * np.eye(NC)
    PtP = P.T @ P
    H = np.linalg.inv(G + LAM_N * SBAR * PtP)
    A18 = A[:, IDX18].T @ A[:, IDX18] + 0.01 * np.eye(18)
    Minit18 = A[:, IDX18] @ np.linalg.inv(A18)  # [300, 18] (A18 symmetric)
    Minit = np.zeros((D, NC), np.float64)
    Minit[:, :16] = Minit18[:, :16]
    Minit[:, 45:] = Minit18[:, 16:]
    return {
        "init_aq": _kt_layout(A, 3).astype(np.float32),        # [128,3,47]
        "init_c": _kt_layout(Minit, 3).astype(np.float32),     # [128,3,47]
        "nph": _kt_layout(-LAM_N * (P @ H), 3).astype(np.float32),
        "corr": (LAM_N * SBAR * (PtP @ H)).astype(np.float32),  # [47,47]
        "hmat": H.astype(np.float32),                           # [47,47]
    }


def _prep_bvox(b_block):
    """b_block [14,14,20,300] (x,y,z,p) -> [128, 3, 3920] with voxel order
    (z, x, y) and the 300-dim k-tiled on partitions."""
    bt = np.asarray(b_block, np.float32).transpose(2, 0, 1, 3).reshape(NVOX0, D).T
    out = np.zeros((DPAD, NVOX0), np.float32)
    out[:D] = bt
    return np.ascontiguousarray(out.reshape(3, 128, NVOX0).transpose(1, 0, 2))


# --------------------------------------------------------------------------
# device program
# --------------------------------------------------------------------------

def build_program():
    nc = bacc.Bacc("TRN2", target_bir_lowering=False, debug=False, num_devices=8)

    # ---- DRAM I/O ----
    bvt = nc.dram_tensor("bvt", [128, 3, NVOX0], F32R, kind="ExternalInput")
    init_aq = nc.dram_tensor("init_aq", [128, 3, NC], F32R, kind="ExternalInput")
    init_c = nc.dram_tensor("init_c", [128, 3, NC], F32R, kind="ExternalInput")
    pt_d = nc.dram_tensor("pt", [NC, D], F32R, kind="ExternalInput")
    nph_d = nc.dram_tensor("nph", [128, 3, NC], F32R, kind="ExternalInput")
    corr_d = nc.dram_tensor("corr", [NC, NC], F32R, kind="ExternalInput")
    hmat_d = nc.dram_tensor("hmat", [NC, NC], F32R, kind="ExternalInput")
    wdram = {}
    for ci in range(4):
        wdram[f"w94_c{ci}"] = nc.dram_tensor(f"w94_c{ci}", [94, 9, 512], F32R, kind="ExternalInput")
        wdram[f"w47_c{ci}"] = nc.dram_tensor(f"w47_c{ci}", [NC, 9, 512], F32R, kind="ExternalInput")
        for nm in ("w2", "w3", "w4"):
            wdram[f"{nm}_c{ci}"] = nc.dram_tensor(f"{nm}_c{ci}", [128, 4, 512], F32R, kind="ExternalInput")
        wdram[f"w5_c{ci}"] = nc.dram_tensor(f"w5_c{ci}", [128, 4, NC], F32R, kind="ExternalInput")
        for nm in ("b1", "b2", "b3", "b4"):
            wdram[f"{nm}_c{ci}"] = nc.dram_tensor(f"{nm}_c{ci}", [128, 4], F32, kind="ExternalInput")
        wdram[f"b5_c{ci}"] = nc.dram_tensor(f"b5_c{ci}", [NC, 1], F32, kind="ExternalInput")
    out_d = nc.dram_tensor("out", [NC, 432], F32, kind="ExternalOutput")

    from contextlib import ExitStack
    with tile.TileContext(nc) as tc, ExitStack() as ctx:
        consts = ctx.enter_context(tc.tile_pool(name="consts", bufs=1))
        wpool1 = ctx.enter_context(tc.tile_pool(name="wpool1", bufs=2))
        wpool = ctx.enter_context(tc.tile_pool(name="wpool", bufs=1))
        bpool = ctx.enter_context(tc.tile_pool(name="bpool", bufs=2))
        hpool = ctx.enter_context(tc.tile_pool(name="hpool", bufs=2))
        upool = ctx.enter_context(tc.tile_pool(name="upool", bufs=1))
        s2pool = ctx.enter_context(tc.tile_pool(name="s2pool", bufs=1))
        smalls = ctx.enter_context(tc.tile_pool(name="smalls", bufs=2))
        cvols = ctx.enter_context(tc.tile_pool(name="cvols", bufs=1))
        pspool = ctx.enter_context(tc.tile_pool(name="pspool", bufs=6, space="PSUM"))
        ps47 = ctx.enter_context(tc.tile_pool(name="ps47", bufs=2, space="PSUM"))

        # ---- persistent constants ----
        pt_sb = consts.tile([NC, D], F32R)
        nc.sync.dma_start(pt_sb[:], pt_d[:])
        nph_sb = consts.tile([128, 3, NC], F32R)
        nc.sync.dma_start(nph_sb[:], nph_d[:])
        corr_sb = consts.tile([NC, NC], F32R)
        nc.sync.dma_start(corr_sb[:], corr_d[:])
        hmat_sb = consts.tile([NC, NC], F32R)
        nc.sync.dma_start(hmat_sb[:], hmat_d[:])
        iaq_sb = consts.tile([128, 3, NC], F32R)
        nc.sync.dma_start(iaq_sb[:], init_aq[:])
        ic_sb = consts.tile([128, 3, NC], F32R)
        nc.sync.dma_start(ic_sb[:], init_c[:])

        # ---- volumes (z-major, doubled partitions for c) ----
        X0, Y0, Z0 = SUB
        aqtb = cvols.tile([NC, Z0, X0, Y0], F32, tag="aqtb")
        c0_t = cvols.tile([94, Z0, X0, Y0], F32R, tag="c0", name="c0_t")
        cvol = {0: c0_t}

        # ---- init: AQ_Tb and c_init over z-slabs ----
        NI = INIT_ZG * X0 * Y0  # 392
        for t in range(Z0 // INIT_ZG):
            v0 = t * NI
            zs = t * INIT_ZG
            bt_sb = hpool.tile([128, 3, NI], F32R, tag="h1", name="bt_sb")
            nc.sync.dma_start(bt_sb[:], bvt[:, :, v0:v0 + NI])
            pa = pspool.tile([NC, NI], F32, tag="ps")
            pc = pspool.tile([NC, NI], F32, tag="ps")
            for kt in range(3):
                nc.tensor.matmul(pa[:], iaq_sb[:, kt, :], bt_sb[:, kt, :],
                                 start=(kt == 0), stop=(kt == 2))
            for kt in range(3):
                nc.tensor.matmul(pc[:], ic_sb[:, kt, :], bt_sb[:, kt, :],
                                 start=(kt == 0), stop=(kt == 2))
            nc.scalar.activation(aqtb[:, zs:zs + INIT_ZG, :, :], pa[:], AF.Copy)
            nc.scalar.activation(c0_t[0:NC, zs:zs + INIT_ZG, :, :], pc[:], AF.Copy)

        # ---- cascades ----
        for n in range(1, 5):
            ci = n - 1
            Xi, Yi, Zi = VOLS[n - 1]
            Xo, Yo, Zo = VOLS[n]
            ZG = ZGRP[n]
            NT = ZG * Xo * Yo
            c_in = cvol[n - 1]
            if n < 4:
                cvol[n] = cvols.tile([94] + [VOLS[n][2], VOLS[n][0], VOLS[n][1]],
                                     F32R, tag=f"c{n}", name=f"cvol{n}")

            # build the z+1-shifted copy on partitions 47..93 of c_in
            nc.sync.dma_start(c_in[NC:2 * NC, 0:Zi - 1, :, :],
                              c_in[0:NC, 1:Zi, :, :])

            # weights for this cascade
            w94 = wpool1.tile([94, 9, 512], F32R, tag="w94")
            nc.sync.dma_start(w94[:], wdram[f"w94_c{ci}"][:])
            w47 = wpool1.tile([NC, 9, 512], F32R, tag="w47")
            nc.sync.dma_start(w47[:], wdram[f"w47_c{ci}"][:])
            w2 = wpool.tile([128, 4, 512], F32R, tag="w2")
            nc.sync.dma_start(w2[:], wdram[f"w2_c{ci}"][:])
            w3 = wpool.tile([128, 4, 512], F32R, tag="w3")
            nc.sync.dma_start(w3[:], wdram[f"w3_c{ci}"][:])
            w4 = wpool.tile([128, 4, 512], F32R, tag="w4")
            nc.sync.dma_start(w4[:], wdram[f"w4_c{ci}"][:])
            w5 = wpool.tile([128, 4, NC], F32R, tag="w5")
            nc.sync.dma_start(w5[:], wdram[f"w5_c{ci}"][:])
            bs = {}
            for nm in ("b1", "b2", "b3", "b4"):
                bs[nm] = bpool.tile([128, 4], F32, tag=nm, name=f"{nm}_{ci}")
                nc.sync.dma_start(bs[nm][:], wdram[f"{nm}_c{ci}"][:])
            b5 = bpool.tile([NC, 1], F32, tag="b5")
            nc.sync.dma_start(b5[:], wdram[f"b5_c{ci}"][:])

            for ti in range(Zo // ZG):
                zs = ti * ZG
                # ---------- MLP layer 1: direct strided views ----------
                h1 = hpool.tile([128, 4, NT], F32R, tag="h1")
                for mt in range(4):
                    pm = pspool.tile([128, NT], F32, tag="ps")
                    for g in range(9):
                        dx, dy = g // 3, g % 3
                        rhs94 = c_in[0:94, zs:zs + ZG,
                                     dx:dx + Xo, dy:dy + Yo]
                        nc.tensor.matmul(pm[:], w94[:, g, mt * 128:(mt + 1) * 128],
                                         rhs94, start=(g == 0), stop=False)
                        rhs47 = c_in[0:NC, zs + 2:zs + 2 + ZG,
                                     dx:dx + Xo, dy:dy + Yo]
                        nc.tensor.matmul(pm[:], w47[:, g, mt * 128:(mt + 1) * 128],
                                         rhs47, start=False, stop=(g == 8))
                    nc.scalar.activation(h1[:, mt, :], pm[:], AF.Relu,
                                         bias=bs["b1"][:, mt:mt + 1])
                hprev = h1
                for li, (wt, bname) in enumerate(
                        ((w2, "b2"), (w3, "b3"), (w4, "b4"))):
                    hnext = hpool.tile([128, 4, NT], F32R,
                                       tag="h2" if li % 2 == 0 else "h1")
                    for mt in range(4):
                        pm = pspool.tile([128, NT], F32, tag="ps")
                        for kt in range(4):
                            nc.tensor.matmul(
                                pm[:], wt[:, kt, mt * 128:(mt + 1) * 128],
                                hprev[:, kt, :], start=(kt == 0), stop=(kt == 3))
                        nc.scalar.activation(hnext[:, mt, :], pm[:], AF.Relu,
                                             bias=bs[bname][:, mt:mt + 1])
                    hprev = hnext
                pw = ps47.tile([NC, NT], F32, tag="ps47")
                for kt in range(4):
                    nc.tensor.matmul(pw[:], w5[:, kt, :], hprev[:, kt, :],
                                     start=(kt == 0), stop=(kt == 3))
                # ---------- build rhs and current c ----------
                rhs_sb = smalls.tile([NC, NT], F32R, tag="rhs")
                nc.scalar.activation(rhs_sb[:], pw[:], AF.Identity, bias=b5[:, 0:1])
                c_view = c_in[0:NC, zs + 1:zs + 1 + ZG, 1:1 + Xo, 1:1 + Yo]
                if n > 1:
                    nc.vector.tensor_add(rhs_sb[:], rhs_sb[:], c_view)
                aq_view = aqtb[:, n + zs:n + zs + ZG, n:n + Xo, n:n + Yo]
                nc.vector.tensor_add(rhs_sb[:], rhs_sb[:], aq_view)
                c_sb = smalls.tile([NC, NT], F32R, tag="cnew")
                nc.vector.tensor_copy(c_sb[:], c_view)
                # ---------- 3 data-consistency iterations ----------
                for it in range(3):
                    s2t = s2pool.tile([128, 3, NT], F32, tag="s2")
                    for mi, (m0, ml) in enumerate(MSEG):
                        zp = pspool.tile([128, NT], F32, tag="ps")
                        nc.tensor.matmul(zp[0:ml, :], pt_sb[:, m0:m0 + ml],
                                         c_sb[:], start=True, stop=True)
                        nc.scalar.activation(s2t[0:ml, mi, :], zp[0:ml, :],
                                             AF.Sigmoid, scale=-ALPHA)
                        nc.scalar.activation(s2t[0:ml, mi, :], s2t[0:ml, mi, :],
                                             AF.Square)
                    tp = ps47.tile([NC, NT], F32, tag="ps47")
                    nc.tensor.matmul(tp[:], hmat_sb[:], rhs_sb[:],
                                     start=True, stop=True)
                    t_sb = smalls.tile([NC, NT], F32R, tag="t")
                    nc.scalar.activation(t_sb[:], tp[:], AF.Copy)
                    cn_sb = smalls.tile([NC, NT], F32R, tag="cnew")
                    nc.vector.tensor_copy(cn_sb[:], t_sb[:])
                    for k in range(KTERMS):
                        u_sb = upool.tile([128, 3, NT], F32R, tag="u")
                        for mi, (m0, ml) in enumerate(MSEG):
                            zp = pspool.tile([128, NT], F32, tag="ps")
                            nc.tensor.matmul(zp[0:ml, :], pt_sb[:, m0:m0 + ml],
                                             t_sb[:], start=True, stop=True)
                            nc.vector.tensor_mul(u_sb[0:ml, mi, :], zp[0:ml, :],
                                                 s2t[0:ml, mi, :])
                        tp2 = ps47.tile([NC, NT], F32, tag="ps47")
                        for ki, (k0, kl) in enumerate(KSEG):
                            nc.tensor.matmul(tp2[:], nph_sb[0:kl, ki, :],
                                             u_sb[0:kl, ki, :],
                                             start=(ki == 0), stop=False)
                        nc.tensor.matmul(tp2[:], corr_sb[:], t_sb[:],
                                         start=False, stop=True)
                        t_sb = smalls.tile([NC, NT], F32R, tag="t")
                        nc.scalar.activation(t_sb[:], tp2[:], AF.Copy)
                        nc.vector.tensor_add(cn_sb[:], cn_sb[:], t_sb[:])
                    c_sb = cn_sb
                # ---------- write result ----------
                if n < 4:
                    nc.vector.tensor_copy(cvol[n][0:NC, zs:zs + ZG, :, :], c_sb[:])
                else:
                    nc.sync.dma_start(out_d[:], c_sb[:].bitcast(F32))

    nc.compile()
    return nc


# --------------------------------------------------------------------------
# entry point
# --------------------------------------------------------------------------

def kernel(b, AQ, P, params, lambda_deep=1.0, lambda_neg=0.1, alpha=50.0):
    global LAST_RESULTS
    b = np.asarray(b, np.float32)
    AQ = np.asarray(AQ, np.float32)
    P_np = np.asarray(P, np.float32)

    shared = _prep_shared(P_np, params)
    batch = [_prep_batch(AQ[bi], P_np) for bi in range(B)]

    if "nc" not in _CACHE:
        _CACHE["nc"] = build_program()
    nc = _CACHE["nc"]

    in_maps = []
    coords = []
    for bi in range(B):
        for qx in range(2):
            for qy in range(2):
                x0, y0 = 6 * qx, 6 * qy
                blk = b[bi, x0:x0 + 14, y0:y0 + 14, :, :, 0]
                m = dict(shared)
                m.update(batch[bi])
                m["bvt"] = _prep_bvox(blk)
                in_maps.append(m)
                coords.append((bi, qx, qy))

    res = run_bass_kernel_spmd(nc, in_maps, core_ids=list(range(8)), trace=TRACE)
    LAST_RESULTS = res

    out = np.zeros((B, 12, 12, 12, NC, 1), np.float32)
    for i, (bi, qx, qy) in enumerate(coords):
        blk = res.results[i]["out"].reshape(NC, 12, 6, 6).transpose(2, 3, 1, 0)
        out[bi, 6 * qx:6 * qx + 6, 6 * qy:6 * qy + 6, :, :, 0] = blk
    return out
